# revision 8
# baseline (speedup 1.0000x reference)
"""Trainium2 Bass kernel for an AttentionBlock (GroupNorm + single-head-dim
self-attention + proj + residual), data-parallel over batch on 8 NeuronCores.

Reference semantics (per batch element, x: [C=512, H=32, W=32], n = H*W = 1024):
  h   = GroupNorm32(x) * scale + bias
  q   = Wq h + bq ; k = Wk h + bk ; v = Wv h + bv     (1x1 convs, [C, n])
  S_h = q_h^T k_h / sqrt(64)   per head h (8 heads, d=64)
  A_h = softmax(S_h)           (over keys)
  o_h = v_h A_h^T
  y   = x + Wp o + bp

Sharding: batch 16 -> 2 per core, fully independent (no collectives).
"""

import numpy as np

import concourse.bacc as bacc
import concourse.bass as bass
import concourse.tile as tile
from concourse import mybir
from concourse.bass_utils import run_bass_kernel_spmd

F32 = mybir.dt.float32
F32R = mybir.dt.float32r
BF16 = mybir.dt.bfloat16
AF = mybir.ActivationFunctionType
OP = mybir.AluOpType

C = 512
NH = 8
D = 64
N = 1024
GROUPS = 32
GS = C // GROUPS  # 16 channels per group
EPS = 1e-5
B_PER_CORE = 2
N_CORES = 8

CT = 4   # c tiles of 128
NT = 8   # n tiles of 128
NCH = 2  # n chunks of 512
VG = 66  # vT per-head group stride (64 data + 1 ones + 1 pad)

E_BUFS = 10


def _bcast_rows(row_ap, parts):
    """Broadcast a single-row (DRAM) AP across `parts` partitions."""
    ap = [[0, parts]] + [list(d) for d in row_ap.ap]
    return bass.AP(tensor=row_ap.tensor, offset=row_ap.offset, ap=ap)


def build_nc(apply_vb, dump=False):
    nc = bacc.Bacc()

    x_ext = nc.declare_dram_parameter("x", [B_PER_CORE, C, N], F32, isOutput=False)
    w_ext = {}
    b_ext = {}
    for nm in ("q", "k", "v", "p"):
        w_ext[nm] = nc.declare_dram_parameter(f"{nm}wT", [C, C], F32, isOutput=False)
        b_ext[nm] = nc.declare_dram_parameter(f"{nm}b", [C], F32, isOutput=False)
    nsc_ext = nc.declare_dram_parameter("nsc", [C], F32, isOutput=False)
    nbi_ext = nc.declare_dram_parameter("nbi", [C], F32, isOutput=False)
    out_ext = nc.declare_dram_parameter("out", [B_PER_CORE, C, N], F32, isOutput=True)

    zdram = nc.dram_tensor("zscratch", [B_PER_CORE, NH, N], BF16)
    gdram = nc.dram_tensor("gscratch", [B_PER_CORE, GROUPS, 2], F32)
    dbg_ext = None
    if dump:
        dbg_ext = nc.declare_dram_parameter("dbg", [10, 128, 4352], F32, isOutput=True)

    with tile.TileContext(nc) as tc:
        with (
            tc.tile_pool(name="const", bufs=1) as const,
            tc.tile_pool(name="work", bufs=1) as work,
            tc.tile_pool(name="xpool", bufs=2) as xpool,
            tc.tile_pool(name="epool", bufs=E_BUFS) as epool,
            tc.tile_pool(name="small", bufs=2) as small,
            tc.tile_pool(name="ps1", bufs=2, space="PSUM") as ps1,
            tc.tile_pool(name="ps2", bufs=2, space="PSUM") as ps2,
        ):
            # ---- persistent weight / bias tiles -------------------------
            w_sb = {}
            for nm in ("q", "k", "v", "p"):
                w_sb[nm] = const.tile([128, CT, C], BF16, name=f"w_{nm}")
                nc.gpsimd.dma_start(
                    out=w_sb[nm],
                    in_=w_ext[nm].ap().rearrange("(kt p) c -> p kt c", p=128),
                )
            bias_sb = {}
            for nm in ("q", "k", "p"):
                bias_sb[nm] = const.tile([128, CT], F32, name=f"b_{nm}")
                nc.sync.dma_start(
                    out=bias_sb[nm],
                    in_=b_ext[nm].ap().rearrange("(ct p) -> p ct", p=128),
                )
            nsc_sb = const.tile([128, CT], F32)
            nc.sync.dma_start(
                out=nsc_sb, in_=nsc_ext.ap().rearrange("(ct p) -> p ct", p=128)
            )
            nbi_sb = const.tile([128, CT], F32)
            nc.sync.dma_start(
                out=nbi_sb, in_=nbi_ext.ap().rearrange("(ct p) -> p ct", p=128)
            )
            if apply_vb:
                vb_bc = const.tile([128, C], F32)
                nc.sync.dma_start(out=vb_bc, in_=_bcast_rows(b_ext["v"].ap(), 128))

            # ---- persistent per-batch working tiles (serial reuse) ------
            h_sb = work.tile([128, CT, N], BF16)
            q_sb = work.tile([128, CT, N], BF16)
            k_sb = work.tile([128, CT, N], BF16)
            att_sb = work.tile([128, CT, N], BF16)
            vt_sb = work.tile([128, NT, NH, VG], BF16)
            tmp_sb = work.tile([128, CT, N], F32)
            # ones column for the Z (softmax denominator) rows; the pad
            # column and data columns are (re)written elsewhere.
            nc.vector.memset(vt_sb[:, :, :, D : D + 1], 1.0)

            for b in range(B_PER_CORE):
                # ==== stage A: load x, GroupNorm stats + apply ==========
                x_sb = xpool.tile([128, CT, N], F32, tag="x")
                xv = x_ext.ap()[b].rearrange("(ct p) n -> p ct n", p=128)
                for ct in range(CT):
                    nc.sync.dma_start(out=x_sb[:, ct, :], in_=xv[:, ct, :])

                cstats = small.tile([128, CT, 2, 6], F32, tag="cstats")
                for ct in range(CT):
                    for s in range(2):
                        nc.vector.bn_stats(
                            out=cstats[:, ct, s, :],
                            in_=x_sb[:, ct, s * 512 : (s + 1) * 512],
                        )
                gstats = small.tile([GROUPS, 2 * GS, 6], F32, tag="gstats")
                for ct in range(CT):
                    for gl in range(8):
                        g = ct * 8 + gl
                        nc.sync.dma_start(
                            out=gstats[g : g + 1, :, :],
                            in_=cstats[gl * GS : (gl + 1) * GS, ct, :, :],
                        )
                gmv = small.tile([GROUPS, 2], F32, tag="gmv")
                nc.vector.bn_aggr(out=gmv, in_=gstats)
                # rstd = exp(-0.5 * ln(var + eps))  (stays in the exp/ln
                # activation-table set; Sqrt would force a table swap)
                eps_t = small.tile([GROUPS, 1], F32, tag="eps")
                nc.vector.memset(eps_t, EPS)
                lnv = small.tile([GROUPS, 1], F32, tag="lnv")
                nc.scalar.activation(out=lnv, in_=gmv[:, 1:2], func=AF.Ln, bias=eps_t)
                nc.scalar.activation(out=gmv[:, 1:2], in_=lnv, func=AF.Exp, scale=-0.5)
                nc.sync.dma_start(out=gdram.ap()[b], in_=gmv)
                cmv = small.tile([128, CT, 2], F32, tag="cmv")
                for ct in range(CT):
                    for gl in range(8):
                        g = ct * 8 + gl
                        nc.sync.dma_start(
                            out=cmv[gl * GS : (gl + 1) * GS, ct, :],
                            in_=_bcast_rows(gdram.ap()[b][g], GS),
                        )
                csr = small.tile([128, CT], F32, tag="csr")
                nc.vector.tensor_mul(out=csr, in0=cmv[:, :, 1], in1=nsc_sb)
                cb2 = small.tile([128, CT], F32, tag="cb2")
                nc.vector.tensor_mul(out=cb2, in0=cmv[:, :, 0], in1=csr)
                nc.vector.tensor_sub(out=cb2, in0=nbi_sb, in1=cb2)
                for ct in range(CT):
                    nc.vector.tensor_scalar(
                        out=h_sb[:, ct, :],
                        in0=x_sb[:, ct, :],
                        scalar1=csr[:, ct : ct + 1],
                        scalar2=cb2[:, ct : ct + 1],
                        op0=OP.mult,
                        op1=OP.add,
                    )

                if dump and b == 0:
                    nc.gpsimd.dma_start(
                        out=dbg_ext.ap()[0][:, 0:4096],
                        in_=h_sb.rearrange("p a n -> p (a n)"),
                    )
                    nc.sync.dma_start(
                        out=dbg_ext.ap()[6][:, 0:8],
                        in_=cmv.rearrange("p a t -> p (a t)"),
                    )
                # ==== stage B: q / k / vT convolutions ==================
                for nm, dst in (("q", q_sb), ("k", k_sb)):
                    for ct in range(CT):
                        ps = ps2.tile([128, N], F32, tag="ps2", name=f"ps_{nm}{ct}")
                        for ch in range(NCH):
                            for kt in range(CT):
                                nc.tensor.matmul(
                                    out=ps[:, ch * 512 : (ch + 1) * 512],
                                    lhsT=w_sb[nm][:, kt, ct * 128 : (ct + 1) * 128],
                                    rhs=h_sb[:, kt, ch * 512 : (ch + 1) * 512],
                                    start=(kt == 0),
                                    stop=(kt == CT - 1),
                                )
                        nc.vector.tensor_scalar(
                            out=dst[:, ct, :],
                            in0=ps,
                            scalar1=bias_sb[nm][:, ct : ct + 1],
                            scalar2=None,
                            op0=OP.add,
                        )
                for nt in range(NT):
                    ps = ps2.tile([128, N], F32, tag="ps2", name=f"ps_v{nt}")
                    for kt in range(CT):
                        nc.tensor.matmul(
                            out=ps[:, 0:512],
                            lhsT=h_sb[:, kt, nt * 128 : (nt + 1) * 128],
                            rhs=w_sb["v"][:, kt, :],
                            start=(kt == 0),
                            stop=(kt == CT - 1),
                        )
                    psv = ps[:, 0:512].rearrange("p (h d) -> p h d", d=D)
                    if apply_vb:
                        nc.vector.tensor_add(
                            out=vt_sb[:, nt, :, 0:D],
                            in0=psv,
                            in1=vb_bc.rearrange("p (h d) -> p h d", d=D),
                        )
                    else:
                        nc.vector.tensor_copy(out=vt_sb[:, nt, :, 0:D], in_=psv)

                if dump and b == 0:
                    nc.gpsimd.dma_start(
                        out=dbg_ext.ap()[1][:, 0:4096],
                        in_=q_sb.rearrange("p a n -> p (a n)"),
                    )
                    nc.gpsimd.dma_start(
                        out=dbg_ext.ap()[2][:, 0:4096],
                        in_=k_sb.rearrange("p a n -> p (a n)"),
                    )
                    nc.gpsimd.dma_start(
                        out=dbg_ext.ap()[4][:, 0 : NT * NH * VG],
                        in_=vt_sb.rearrange("p a h g -> p (a h g)"),
                    )
                # ==== stage C: attention ================================
                # compute-engine APs may only start at partition 0/32/64/96,
                # so Z rows land as columns of a single-partition tile and a
                # DMA respreads them across partitions for the reciprocal.
                zflat = small.tile([1, NH * N], BF16, tag="zflat", bufs=1)
                zrows = small.tile([NH, N], F32, tag="zrows", bufs=1)
                for hp in range(CT):  # head pair = c tile of q/k
                    e_tiles = []
                    for mt in range(NT):
                        e_t = epool.tile(
                            [128, 2, N], BF16, tag="e", name=f"e{hp}_{mt}"
                        )
                        for hi, p0 in ((0, 0), (1, 64)):
                            psS = ps1.tile(
                                [128, N], F32, tag="ps1", name=f"psS{hp}_{mt}_{hi}"
                            )
                            for ch in range(NCH):
                                nc.tensor.matmul(
                                    out=psS[:, ch * 512 : (ch + 1) * 512],
                                    lhsT=k_sb[
                                        p0 : p0 + D, hp, mt * 128 : (mt + 1) * 128
                                    ],
                                    rhs=q_sb[p0 : p0 + D, hp, ch * 512 : (ch + 1) * 512],
                                    start=True,
                                    stop=True,
                                    tile_position=(p0, 0),
                                )
                            nc.scalar.activation(
                                out=e_t[:, hi, :], in_=psS, func=AF.Exp, scale=0.125
                            )
                        e_tiles.append(e_t)
                    for hi, p0 in ((0, 0), (1, 64)):
                        h_ = 2 * hp + hi
                        pso = ps2.tile([128, N], F32, tag="ps2", name=f"psO{hp}_{hi}")
                        for ch in range(NCH):
                            for mt in range(NT):
                                nc.tensor.matmul(
                                    out=pso[0 : D + 1, ch * 512 : (ch + 1) * 512],
                                    lhsT=vt_sb[:, mt, h_, 0 : D + 1],
                                    rhs=e_tiles[mt][:, hi, ch * 512 : (ch + 1) * 512],
                                    start=(mt == 0),
                                    stop=(mt == NT - 1),
                                )
                        nc.vector.tensor_copy(
                            out=att_sb[p0 : p0 + D, hp, :], in_=pso[0:D, :]
                        )
                        nc.vector.tensor_copy(
                            out=zflat[0:1, h_ * N : (h_ + 1) * N],
                            in_=pso[D : D + 1, :],
                        )
                nc.gpsimd.dma_start(
                    out=zrows, in_=zflat
                )
                rzt = small.tile([NH, N], F32, tag="rzt", bufs=1)
                nc.vector.reciprocal_approx_fast(out=rzt, in_=zrows)
                nc.gpsimd.dma_start(
                    out=zdram.ap()[b].bitcast(BF16), in_=rzt
                )
                if dump and b == 0:
                    nc.sync.dma_start(out=dbg_ext.ap()[5][0:8, 0:N], in_=zrows)
                    nc.sync.dma_start(out=dbg_ext.ap()[7][0:8, 0:N], in_=rzt)
                    nc.gpsimd.dma_start(
                        out=dbg_ext.ap()[8][:, 0:4096],
                        in_=att_sb.rearrange("p a n -> p (a n)"),
                    )
                for hp in range(CT):
                    rzb = small.tile([128, N], BF16, tag="rzb")
                    for hi, p0 in ((0, 0), (1, 64)):
                        nc.sync.dma_start(
                            out=rzb[p0 : p0 + D, :],
                            in_=_bcast_rows(zdram.ap()[b][2 * hp + hi], D),
                        )
                    nc.vector.tensor_mul(
                        out=att_sb[:, hp, :], in0=att_sb[:, hp, :], in1=rzb
                    )

                if dump and b == 0:
                    nc.gpsimd.dma_start(
                        out=dbg_ext.ap()[3][:, 0:4096],
                        in_=att_sb.rearrange("p a n -> p (a n)"),
                    )
                # ==== stage D: proj + residual ==========================
                ov = out_ext.ap()[b].rearrange("(ct p) n -> p ct n", p=128)
                for ct in range(CT):
                    ps = ps2.tile([128, N], F32, tag="ps2", name=f"ps_p{ct}")
                    for ch in range(NCH):
                        for kt in range(CT):
                            nc.tensor.matmul(
                                out=ps[:, ch * 512 : (ch + 1) * 512],
                                lhsT=w_sb["p"][:, kt, ct * 128 : (ct + 1) * 128],
                                rhs=att_sb[:, kt, ch * 512 : (ch + 1) * 512],
                                start=(kt == 0),
                                stop=(kt == CT - 1),
                            )
                    nc.vector.tensor_scalar(
                        out=tmp_sb[:, ct, :],
                        in0=ps,
                        scalar1=bias_sb["p"][:, ct : ct + 1],
                        scalar2=None,
                        op0=OP.add,
                    )
                    nc.vector.tensor_add(
                        out=x_sb[:, ct, :], in0=tmp_sb[:, ct, :], in1=x_sb[:, ct, :]
                    )
                    nc.sync.dma_start(out=ov[:, ct, :], in_=x_sb[:, ct, :])

    nc.compile()
    return nc


def kernel(x, norm_scale, norm_bias, q_w, q_b, k_w, k_b, v_w, v_b, proj_w, proj_b,
           _dump=False):
    x = np.ascontiguousarray(np.asarray(x, dtype=np.float32))
    b, c, hh, ww = x.shape
    assert (b, c, hh * ww) == (16, C, N)
    xr = x.reshape(b, c, hh * ww)

    wts = {
        "qwT": np.ascontiguousarray(np.asarray(q_w, np.float32).T),
        "kwT": np.ascontiguousarray(np.asarray(k_w, np.float32).T),
        "vwT": np.ascontiguousarray(np.asarray(v_w, np.float32).T),
        "pwT": np.ascontiguousarray(np.asarray(proj_w, np.float32).T),
        "qb": np.ascontiguousarray(np.asarray(q_b, np.float32)),
        "kb": np.ascontiguousarray(np.asarray(k_b, np.float32)),
        "vb": np.ascontiguousarray(np.asarray(v_b, np.float32)),
        "pb": np.ascontiguousarray(np.asarray(proj_b, np.float32)),
        "nsc": np.ascontiguousarray(np.asarray(norm_scale, np.float32)),
        "nbi": np.ascontiguousarray(np.asarray(norm_bias, np.float32)),
    }
    apply_vb = bool(np.any(wts["vb"]))

    nc = build_nc(apply_vb, dump=_dump)
    in_maps = []
    for i in range(N_CORES):
        m = dict(wts)
        m["x"] = np.ascontiguousarray(xr[i * B_PER_CORE : (i + 1) * B_PER_CORE])
        in_maps.append(m)

    res = run_bass_kernel_spmd(nc, in_maps, core_ids=list(range(N_CORES)))
    kernel.last_result = res
    out = np.concatenate([res.results[i]["out"] for i in range(N_CORES)], axis=0)
    return out.reshape(b, c, hh, ww).astype(np.float32)


# revision 9
# speedup vs baseline: 1.0641x; 1.0641x over previous
"""Trainium2 Bass kernel for an AttentionBlock (GroupNorm + single-head-dim
self-attention + proj + residual), data-parallel over batch on 8 NeuronCores.

Reference semantics (per batch element, x: [C=512, H=32, W=32], n = H*W = 1024):
  h   = GroupNorm32(x) * scale + bias
  q   = Wq h + bq ; k = Wk h + bk ; v = Wv h + bv     (1x1 convs, [C, n])
  S_h = q_h^T k_h / sqrt(64)   per head h (8 heads, d=64)
  A_h = softmax(S_h)           (over keys)
  o_h = v_h A_h^T
  y   = x + Wp o + bp

Sharding: batch 16 -> 2 per core, fully independent (no collectives).
"""

import numpy as np

import concourse.bacc as bacc
import concourse.bass as bass
import concourse.tile as tile
from concourse import mybir
from concourse.bass_utils import run_bass_kernel_spmd

F32 = mybir.dt.float32
F32R = mybir.dt.float32r
BF16 = mybir.dt.bfloat16
AF = mybir.ActivationFunctionType
OP = mybir.AluOpType

C = 512
NH = 8
D = 64
N = 1024
GROUPS = 32
GS = C // GROUPS  # 16 channels per group
EPS = 1e-5
B_PER_CORE = 2
N_CORES = 8

CT = 4   # c tiles of 128
NT = 8   # n tiles of 128
NCH = 2  # n chunks of 512
VG = 66  # vT per-head group stride (64 data + 1 ones + 1 pad)

E_BUFS = 10


def _bcast_rows(row_ap, parts):
    """Broadcast a single-row (DRAM) AP across `parts` partitions."""
    ap = [[0, parts]] + [list(d) for d in row_ap.ap]
    return bass.AP(tensor=row_ap.tensor, offset=row_ap.offset, ap=ap)


def build_nc(apply_vb, dump=False):
    nc = bacc.Bacc()

    x_ext = nc.declare_dram_parameter("x", [B_PER_CORE, C, N], F32, isOutput=False)
    w_ext = {}
    b_ext = {}
    for nm in ("q", "k", "v", "p"):
        w_ext[nm] = nc.declare_dram_parameter(f"{nm}wT", [C, C], F32, isOutput=False)
        b_ext[nm] = nc.declare_dram_parameter(f"{nm}b", [C], F32, isOutput=False)
    nsc_ext = nc.declare_dram_parameter("nsc", [C], F32, isOutput=False)
    nbi_ext = nc.declare_dram_parameter("nbi", [C], F32, isOutput=False)
    out_ext = nc.declare_dram_parameter("out", [B_PER_CORE, C, N], F32, isOutput=True)

    zdram = nc.dram_tensor("zscratch", [B_PER_CORE, NH, N], BF16)
    gdram = nc.dram_tensor("gscratch", [B_PER_CORE, GROUPS, 2], F32)
    dbg_ext = None
    if dump:
        dbg_ext = nc.declare_dram_parameter("dbg", [10, 128, 4352], F32, isOutput=True)

    with tile.TileContext(nc) as tc:
        with (
            tc.tile_pool(name="const", bufs=1) as const,
            tc.tile_pool(name="work", bufs=2) as work,
            tc.tile_pool(name="xpool", bufs=2) as xpool,
            tc.tile_pool(name="epool", bufs=E_BUFS) as epool,
            tc.tile_pool(name="small", bufs=2) as small,
            tc.tile_pool(name="ps1", bufs=1, space="PSUM") as ps1,
            tc.tile_pool(name="ps2", bufs=2, space="PSUM") as ps2,
        ):
            # ---- persistent weight / bias tiles -------------------------
            w_sb = {}
            for nm in ("q", "k", "v", "p"):
                w_sb[nm] = const.tile([128, CT, C], BF16, name=f"w_{nm}")
                nc.gpsimd.dma_start(
                    out=w_sb[nm],
                    in_=w_ext[nm].ap().rearrange("(kt p) c -> p kt c", p=128),
                )
            bias_sb = {}
            for nm in ("q", "k", "p"):
                bias_sb[nm] = const.tile([128, CT], F32, name=f"b_{nm}")
                nc.sync.dma_start(
                    out=bias_sb[nm],
                    in_=b_ext[nm].ap().rearrange("(ct p) -> p ct", p=128),
                )
            nsc_sb = const.tile([128, CT], F32)
            nc.sync.dma_start(
                out=nsc_sb, in_=nsc_ext.ap().rearrange("(ct p) -> p ct", p=128)
            )
            nbi_sb = const.tile([128, CT], F32)
            nc.sync.dma_start(
                out=nbi_sb, in_=nbi_ext.ap().rearrange("(ct p) -> p ct", p=128)
            )
            vb_bc = None
            if apply_vb:
                vb_bc = const.tile([128, C], F32)
                nc.sync.dma_start(out=vb_bc, in_=_bcast_rows(b_ext["v"].ap(), 128))
            eps_t = const.tile([GROUPS, 1], F32)
            nc.vector.memset(eps_t, EPS)

            st = {}  # per-batch tile handles

            def emit_A(b):
                """load x, GroupNorm stats + apply -> h"""
                x_sb = xpool.tile([128, CT, N], F32, tag="x", name=f"x{b}")
                h_sb = work.tile([128, CT, N], BF16, tag="h", name=f"h{b}")
                st[b] = {"x": x_sb, "h": h_sb}
                xv = x_ext.ap()[b].rearrange("(ct p) n -> p ct n", p=128)
                for ct in range(CT):
                    nc.sync.dma_start(out=x_sb[:, ct, :], in_=xv[:, ct, :])
                cstats = small.tile([128, CT, 2, 6], F32, tag="cstats")
                for ct in range(CT):
                    for sg in range(2):
                        nc.vector.bn_stats(
                            out=cstats[:, ct, sg, :],
                            in_=x_sb[:, ct, sg * 512 : (sg + 1) * 512],
                        )
                gstats = small.tile([GROUPS, 2 * GS, 6], F32, tag="gstats")
                for ct in range(CT):
                    for gl in range(8):
                        g = ct * 8 + gl
                        nc.sync.dma_start(
                            out=gstats[g : g + 1, :, :],
                            in_=cstats[gl * GS : (gl + 1) * GS, ct, :, :],
                        )
                gmv = small.tile([GROUPS, 2], F32, tag="gmv")
                nc.vector.bn_aggr(out=gmv, in_=gstats)
                # rstd = exp(-0.5 * ln(var + eps)) to stay in the exp/ln
                # activation-table set (Sqrt would force a table swap)
                lnv = small.tile([GROUPS, 1], F32, tag="lnv")
                nc.scalar.activation(out=lnv, in_=gmv[:, 1:2], func=AF.Ln, bias=eps_t)
                nc.scalar.activation(out=gmv[:, 1:2], in_=lnv, func=AF.Exp, scale=-0.5)
                nc.sync.dma_start(out=gdram.ap()[b], in_=gmv)
                cmv = small.tile([128, CT, 2], F32, tag="cmv")
                for ct in range(CT):
                    for gl in range(8):
                        g = ct * 8 + gl
                        nc.sync.dma_start(
                            out=cmv[gl * GS : (gl + 1) * GS, ct, :],
                            in_=_bcast_rows(gdram.ap()[b][g], GS),
                        )
                csr = small.tile([128, CT], F32, tag="csr")
                nc.vector.tensor_mul(out=csr, in0=cmv[:, :, 1], in1=nsc_sb)
                cb2 = small.tile([128, CT], F32, tag="cb2")
                nc.vector.tensor_mul(out=cb2, in0=cmv[:, :, 0], in1=csr)
                nc.vector.tensor_sub(out=cb2, in0=nbi_sb, in1=cb2)
                for ct in range(CT):
                    nc.vector.tensor_scalar(
                        out=h_sb[:, ct, :],
                        in0=x_sb[:, ct, :],
                        scalar1=csr[:, ct : ct + 1],
                        scalar2=cb2[:, ct : ct + 1],
                        op0=OP.mult,
                        op1=OP.add,
                    )
                if dump and b == 0:
                    nc.gpsimd.dma_start(
                        out=dbg_ext.ap()[0][:, 0:4096],
                        in_=h_sb.rearrange("p a n -> p (a n)"),
                    )
                    nc.sync.dma_start(
                        out=dbg_ext.ap()[6][:, 0:8],
                        in_=cmv.rearrange("p a t -> p (a t)"),
                    )

            def emit_B(b):
                """q / k / vT convolutions"""
                h_sb = st[b]["h"]
                q_sb = work.tile([128, CT, N], BF16, tag="q", name=f"q{b}")
                k_sb = work.tile([128, CT, N], BF16, tag="k", name=f"k{b}")
                vt_sb = work.tile([128, NT, NH, VG], BF16, tag="vt", name=f"vt{b}")
                st[b].update({"q": q_sb, "k": k_sb, "vt": vt_sb})
                nc.vector.memset(vt_sb[:, :, :, D : D + 1], 1.0)
                for nm, dst in (("q", q_sb), ("k", k_sb)):
                    for ct in range(CT):
                        ps = ps2.tile([128, N], F32, tag="ps2", name=f"ps_{nm}{ct}")
                        for ch in range(NCH):
                            for kt in range(CT):
                                nc.tensor.matmul(
                                    out=ps[:, ch * 512 : (ch + 1) * 512],
                                    lhsT=w_sb[nm][:, kt, ct * 128 : (ct + 1) * 128],
                                    rhs=h_sb[:, kt, ch * 512 : (ch + 1) * 512],
                                    start=(kt == 0),
                                    stop=(kt == CT - 1),
                                )
                        nc.vector.tensor_scalar(
                            out=dst[:, ct, :],
                            in0=ps,
                            scalar1=bias_sb[nm][:, ct : ct + 1],
                            scalar2=None,
                            op0=OP.add,
                        )
                for nt in range(NT):
                    ps = ps2.tile([128, N], F32, tag="ps2", name=f"ps_v{nt}")
                    for kt in range(CT):
                        nc.tensor.matmul(
                            out=ps[:, 0:512],
                            lhsT=h_sb[:, kt, nt * 128 : (nt + 1) * 128],
                            rhs=w_sb["v"][:, kt, :],
                            start=(kt == 0),
                            stop=(kt == CT - 1),
                        )
                    psv = ps[:, 0:512].rearrange("p (h d) -> p h d", d=D)
                    if apply_vb:
                        nc.vector.tensor_add(
                            out=vt_sb[:, nt, :, 0:D],
                            in0=psv,
                            in1=vb_bc.rearrange("p (h d) -> p h d", d=D),
                        )
                    else:
                        nc.vector.tensor_copy(out=vt_sb[:, nt, :, 0:D], in_=psv)

            def emit_C(b):
                """attention"""
                q_sb, k_sb, vt_sb = st[b]["q"], st[b]["k"], st[b]["vt"]
                att_sb = work.tile(
                    [128, CT, N], BF16, tag="att", bufs=1, name=f"att{b}"
                )
                st[b]["att"] = att_sb
                if dump and b == 0:
                    nc.gpsimd.dma_start(
                        out=dbg_ext.ap()[1][:, 0:4096],
                        in_=q_sb.rearrange("p a n -> p (a n)"),
                    )
                    nc.gpsimd.dma_start(
                        out=dbg_ext.ap()[2][:, 0:4096],
                        in_=k_sb.rearrange("p a n -> p (a n)"),
                    )
                    nc.gpsimd.dma_start(
                        out=dbg_ext.ap()[4][:, 0 : NT * NH * VG],
                        in_=vt_sb.rearrange("p a h g -> p (a h g)"),
                    )
                # compute-engine APs may only start at partition 0/32/64/96,
                # so Z rows land as columns of a single-partition tile and a
                # DMA respreads them across partitions for the reciprocal.
                zflat = small.tile([1, NH * N], BF16, tag="zflat", bufs=1)
                zrows = small.tile([NH, N], F32, tag="zrows", bufs=1)
                for hp in range(CT):
                    e_tiles = []
                    for mt in range(NT):
                        psS = ps1.tile([128, 2 * N], F32, tag="ps1", name=f"psS{hp}_{mt}")
                        e_t = epool.tile([128, 2, N], BF16, tag="e", name=f"e{hp}_{mt}")
                        for hi, p0 in ((0, 0), (1, 64)):
                            for ch in range(NCH):
                                nc.tensor.matmul(
                                    out=psS[
                                        :, hi * N + ch * 512 : hi * N + (ch + 1) * 512
                                    ],
                                    lhsT=k_sb[
                                        p0 : p0 + D, hp, mt * 128 : (mt + 1) * 128
                                    ],
                                    rhs=q_sb[p0 : p0 + D, hp, ch * 512 : (ch + 1) * 512],
                                    start=True,
                                    stop=True,
                                    tile_position=(p0, 0),
                                )
                        nc.scalar.activation(
                            out=e_t.rearrange("p a n -> p (a n)"),
                            in_=psS,
                            func=AF.Exp,
                            scale=0.125,
                        )
                        e_tiles.append(e_t)
                    for hi, p0 in ((0, 0), (1, 64)):
                        h_ = 2 * hp + hi
                        pso = ps2.tile([128, N], F32, tag="ps2", name=f"psO{hp}_{hi}")
                        for ch in range(NCH):
                            for mt in range(NT):
                                nc.tensor.matmul(
                                    out=pso[0 : D + 1, ch * 512 : (ch + 1) * 512],
                                    lhsT=vt_sb[:, mt, h_, 0 : D + 1],
                                    rhs=e_tiles[mt][:, hi, ch * 512 : (ch + 1) * 512],
                                    start=(mt == 0),
                                    stop=(mt == NT - 1),
                                )
                        nc.vector.tensor_copy(
                            out=att_sb[p0 : p0 + D, hp, :], in_=pso[0:D, :]
                        )
                        nc.vector.tensor_copy(
                            out=zflat[0:1, h_ * N : (h_ + 1) * N],
                            in_=pso[D : D + 1, :],
                        )
                nc.gpsimd.dma_start(out=zrows, in_=zflat)
                rzt = small.tile([NH, N], F32, tag="rzt", bufs=1)
                nc.vector.reciprocal_approx_fast(out=rzt, in_=zrows)
                nc.gpsimd.dma_start(out=zdram.ap()[b], in_=rzt)
                if dump and b == 0:
                    nc.sync.dma_start(out=dbg_ext.ap()[5][0:8, 0:N], in_=zrows)
                    nc.sync.dma_start(out=dbg_ext.ap()[7][0:8, 0:N], in_=rzt)
                    nc.gpsimd.dma_start(
                        out=dbg_ext.ap()[8][:, 0:4096],
                        in_=att_sb.rearrange("p a n -> p (a n)"),
                    )
                for hp in range(CT):
                    rzb = small.tile([128, N], BF16, tag="rzb")
                    for hi, p0 in ((0, 0), (1, 64)):
                        nc.sync.dma_start(
                            out=rzb[p0 : p0 + D, :],
                            in_=_bcast_rows(zdram.ap()[b][2 * hp + hi], D),
                        )
                    nc.vector.tensor_mul(
                        out=att_sb[:, hp, :], in0=att_sb[:, hp, :], in1=rzb
                    )
                if dump and b == 0:
                    nc.gpsimd.dma_start(
                        out=dbg_ext.ap()[3][:, 0:4096],
                        in_=att_sb.rearrange("p a n -> p (a n)"),
                    )

            def emit_D(b):
                """proj + residual + store"""
                x_sb, att_sb = st[b]["x"], st[b]["att"]
                ov = out_ext.ap()[b].rearrange("(ct p) n -> p ct n", p=128)
                for ct in range(CT):
                    ps = ps2.tile([128, N], F32, tag="ps2", name=f"ps_p{ct}")
                    for ch in range(NCH):
                        for kt in range(CT):
                            nc.tensor.matmul(
                                out=ps[:, ch * 512 : (ch + 1) * 512],
                                lhsT=w_sb["p"][:, kt, ct * 128 : (ct + 1) * 128],
                                rhs=att_sb[:, kt, ch * 512 : (ch + 1) * 512],
                                start=(kt == 0),
                                stop=(kt == CT - 1),
                            )
                    nc.vector.scalar_tensor_tensor(
                        out=x_sb[:, ct, :],
                        in0=ps,
                        scalar=bias_sb["p"][:, ct : ct + 1],
                        in1=x_sb[:, ct, :],
                        op0=OP.add,
                        op1=OP.add,
                    )
                    nc.sync.dma_start(out=ov[:, ct, :], in_=x_sb[:, ct, :])

            emit_A(0)
            emit_B(0)
            emit_A(1)
            emit_C(0)
            emit_B(1)
            emit_D(0)
            emit_C(1)
            emit_D(1)

    nc.compile()
    return nc


def kernel(x, norm_scale, norm_bias, q_w, q_b, k_w, k_b, v_w, v_b, proj_w, proj_b,
           _dump=False):
    x = np.ascontiguousarray(np.asarray(x, dtype=np.float32))
    b, c, hh, ww = x.shape
    assert (b, c, hh * ww) == (16, C, N)
    xr = x.reshape(b, c, hh * ww)

    wts = {
        "qwT": np.ascontiguousarray(np.asarray(q_w, np.float32).T),
        "kwT": np.ascontiguousarray(np.asarray(k_w, np.float32).T),
        "vwT": np.ascontiguousarray(np.asarray(v_w, np.float32).T),
        "pwT": np.ascontiguousarray(np.asarray(proj_w, np.float32).T),
        "qb": np.ascontiguousarray(np.asarray(q_b, np.float32)),
        "kb": np.ascontiguousarray(np.asarray(k_b, np.float32)),
        "vb": np.ascontiguousarray(np.asarray(v_b, np.float32)),
        "pb": np.ascontiguousarray(np.asarray(proj_b, np.float32)),
        "nsc": np.ascontiguousarray(np.asarray(norm_scale, np.float32)),
        "nbi": np.ascontiguousarray(np.asarray(norm_bias, np.float32)),
    }
    apply_vb = bool(np.any(wts["vb"]))

    nc = build_nc(apply_vb, dump=_dump)
    in_maps = []
    for i in range(N_CORES):
        m = dict(wts)
        m["x"] = np.ascontiguousarray(xr[i * B_PER_CORE : (i + 1) * B_PER_CORE])
        in_maps.append(m)

    res = run_bass_kernel_spmd(nc, in_maps, core_ids=list(range(N_CORES)))
    kernel.last_result = res
    out = np.concatenate([res.results[i]["out"] for i in range(N_CORES)], axis=0)
    return out.reshape(b, c, hh, ww).astype(np.float32)


# revision 11
# speedup vs baseline: 1.1289x; 1.0608x over previous
"""Trainium2 Bass kernel for an AttentionBlock (GroupNorm + single-head-dim
self-attention + proj + residual), data-parallel over batch on 8 NeuronCores.

Reference semantics (per batch element, x: [C=512, H=32, W=32], n = H*W = 1024):
  h   = GroupNorm32(x) * scale + bias
  q   = Wq h + bq ; k = Wk h + bk ; v = Wv h + bv     (1x1 convs, [C, n])
  S_h = q_h^T k_h / sqrt(64)   per head h (8 heads, d=64)
  A_h = softmax(S_h)           (over keys)
  o_h = v_h A_h^T
  y   = x + Wp o + bp

Sharding: batch 16 -> 2 per core, fully independent (no collectives).
"""

import numpy as np

import concourse.bacc as bacc
import concourse.bass as bass
import concourse.tile as tile
from concourse import mybir
from concourse.bass_utils import run_bass_kernel_spmd

F32 = mybir.dt.float32
F32R = mybir.dt.float32r
BF16 = mybir.dt.bfloat16
AF = mybir.ActivationFunctionType
OP = mybir.AluOpType

C = 512
NH = 8
D = 64
N = 1024
GROUPS = 32
GS = C // GROUPS  # 16 channels per group
EPS = 1e-5
B_PER_CORE = 2
N_CORES = 8

CT = 4   # c tiles of 128
NT = 8   # n tiles of 128
NCH = 2  # n chunks of 512
VG = 66  # vT per-head group stride (64 data + 1 ones + 1 pad)

E_BUFS = 10


def _bcast_rows(row_ap, parts):
    """Broadcast a single-row (DRAM) AP across `parts` partitions."""
    ap = [[0, parts]] + [list(d) for d in row_ap.ap]
    return bass.AP(tensor=row_ap.tensor, offset=row_ap.offset, ap=ap)


def build_nc(apply_vb, dump=False):
    nc = bacc.Bacc()

    x_ext = nc.declare_dram_parameter("x", [B_PER_CORE, C, N], F32, isOutput=False)
    w_ext = {}
    b_ext = {}
    for nm in ("q", "k", "v", "p"):
        w_ext[nm] = nc.declare_dram_parameter(f"{nm}wT", [C, C], F32, isOutput=False)
        b_ext[nm] = nc.declare_dram_parameter(f"{nm}b", [C], F32, isOutput=False)
    nsc_ext = nc.declare_dram_parameter("nsc", [C], F32, isOutput=False)
    nbi_ext = nc.declare_dram_parameter("nbi", [C], F32, isOutput=False)
    out_ext = nc.declare_dram_parameter("out", [B_PER_CORE, C, N], F32, isOutput=True)

    zdram = nc.dram_tensor("zscratch", [B_PER_CORE, NH, N], BF16)
    gdram = nc.dram_tensor("gscratch", [B_PER_CORE, C, 2], F32)
    cdram = nc.dram_tensor("cscratch", [B_PER_CORE, 128, CT * 12], F32)
    dbg_ext = None
    if dump:
        dbg_ext = nc.declare_dram_parameter("dbg", [10, 128, 4352], F32, isOutput=True)

    with tile.TileContext(nc) as tc:
        with (
            tc.tile_pool(name="const", bufs=1) as const,
            tc.tile_pool(name="work", bufs=2) as work,
            tc.tile_pool(name="xpool", bufs=2) as xpool,
            tc.tile_pool(name="epool", bufs=E_BUFS) as epool,
            tc.tile_pool(name="small", bufs=2) as small,
            tc.tile_pool(name="ps1", bufs=1, space="PSUM") as ps1,
            tc.tile_pool(name="ps2", bufs=2, space="PSUM") as ps2,
        ):
            # ---- persistent weight / bias tiles -------------------------
            w_sb = {}
            for nm in ("q", "k", "v", "p"):
                w_sb[nm] = const.tile([128, CT, C], BF16, name=f"w_{nm}")
                nc.gpsimd.dma_start(
                    out=w_sb[nm],
                    in_=w_ext[nm].ap().rearrange("(kt p) c -> p kt c", p=128),
                )
            bias_sb = {}
            for nm in ("q", "k", "p"):
                bias_sb[nm] = const.tile([128, CT], F32, name=f"b_{nm}")
                nc.sync.dma_start(
                    out=bias_sb[nm],
                    in_=b_ext[nm].ap().rearrange("(ct p) -> p ct", p=128),
                )
            nsc_sb = const.tile([128, CT], F32)
            nc.sync.dma_start(
                out=nsc_sb, in_=nsc_ext.ap().rearrange("(ct p) -> p ct", p=128)
            )
            nbi_sb = const.tile([128, CT], F32)
            nc.sync.dma_start(
                out=nbi_sb, in_=nbi_ext.ap().rearrange("(ct p) -> p ct", p=128)
            )
            vb_bc = None
            if apply_vb:
                vb_bc = const.tile([128, C], F32)
                nc.sync.dma_start(out=vb_bc, in_=_bcast_rows(b_ext["v"].ap(), 128))
            eps_t = const.tile([GROUPS, 1], F32)
            nc.vector.memset(eps_t, EPS)

            st = {}  # per-batch tile handles

            def emit_A(b):
                """load x, GroupNorm stats + apply -> h"""
                x_sb = xpool.tile([128, CT, N], F32, tag="x", name=f"x{b}")
                h_sb = work.tile([128, CT, N], BF16, tag="h", name=f"h{b}")
                st[b] = {"x": x_sb, "h": h_sb}
                xv = x_ext.ap()[b].rearrange("(ct p) n -> p ct n", p=128)
                for ct in range(CT):
                    nc.sync.dma_start(out=x_sb[:, ct, :], in_=xv[:, ct, :])
                cstats = small.tile([128, CT, 2, 6], F32, tag="cstats")
                for ct in range(CT):
                    for sg in range(2):
                        nc.vector.bn_stats(
                            out=cstats[:, ct, sg, :],
                            in_=x_sb[:, ct, sg * 512 : (sg + 1) * 512],
                        )
                nc.sync.dma_start(
                    out=cdram.ap()[b],
                    in_=cstats.rearrange("p ct s f -> p (ct s f)"),
                )
                gstats = small.tile([GROUPS, GS, 2, 6], F32, tag="gstats")
                for ct in range(CT):
                    src = bass.AP(
                        tensor=cdram,
                        offset=b * 128 * CT * 12 + ct * 12,
                        ap=[[GS * CT * 12, 8], [CT * 12, GS], [6, 2], [1, 6]],
                    )
                    nc.sync.dma_start(
                        out=gstats[ct * 8 : (ct + 1) * 8, :, :, :], in_=src
                    )
                gmv = small.tile([GROUPS, 2], F32, tag="gmv")
                nc.vector.bn_aggr(out=gmv, in_=gstats)
                # rstd = exp(-0.5 * ln(var + eps)) to stay in the exp/ln
                # activation-table set (Sqrt would force a table swap)
                lnv = small.tile([GROUPS, 1], F32, tag="lnv")
                nc.scalar.activation(out=lnv, in_=gmv[:, 1:2], func=AF.Ln, bias=eps_t)
                nc.scalar.activation(out=gmv[:, 1:2], in_=lnv, func=AF.Exp, scale=-0.5)
                gmv_rep = bass.AP(
                    tensor=gmv.tensor, offset=gmv.offset,
                    ap=[list(gmv.ap[0]), [0, GS], list(gmv.ap[1])],
                )
                nc.sync.dma_start(out=gdram.ap()[b], in_=gmv_rep)
                cmv = small.tile([128, CT, 2], F32, tag="cmv")
                nc.sync.dma_start(
                    out=cmv,
                    in_=gdram.ap()[b].rearrange("(ct p) s -> p ct s", p=128),
                )
                csr = small.tile([128, CT], F32, tag="csr")
                nc.vector.tensor_mul(out=csr, in0=cmv[:, :, 1], in1=nsc_sb)
                cb2 = small.tile([128, CT], F32, tag="cb2")
                nc.vector.tensor_mul(out=cb2, in0=cmv[:, :, 0], in1=csr)
                nc.vector.tensor_sub(out=cb2, in0=nbi_sb, in1=cb2)
                for ct in range(CT):
                    nc.vector.tensor_scalar(
                        out=h_sb[:, ct, :],
                        in0=x_sb[:, ct, :],
                        scalar1=csr[:, ct : ct + 1],
                        scalar2=cb2[:, ct : ct + 1],
                        op0=OP.mult,
                        op1=OP.add,
                    )
                if dump and b == 0:
                    nc.gpsimd.dma_start(
                        out=dbg_ext.ap()[0][:, 0:4096],
                        in_=h_sb.rearrange("p a n -> p (a n)"),
                    )
                    nc.sync.dma_start(
                        out=dbg_ext.ap()[6][:, 0:8],
                        in_=cmv.rearrange("p a t -> p (a t)"),
                    )

            def emit_B(b):
                """q / k / vT convolutions"""
                h_sb = st[b]["h"]
                q_sb = work.tile([128, CT, N], BF16, tag="q", name=f"q{b}")
                k_sb = work.tile([128, CT, N], BF16, tag="k", name=f"k{b}")
                vt_sb = work.tile([128, NT, NH, VG], BF16, tag="vt", name=f"vt{b}")
                st[b].update({"q": q_sb, "k": k_sb, "vt": vt_sb})
                nc.vector.memset(vt_sb[:, :, :, D : D + 1], 1.0)
                for nm, dst in (("q", q_sb), ("k", k_sb)):
                    for ct in range(CT):
                        ps = ps2.tile([128, N], F32, tag="ps2", name=f"ps_{nm}{ct}")
                        for ch in range(NCH):
                            for kt in range(CT):
                                nc.tensor.matmul(
                                    out=ps[:, ch * 512 : (ch + 1) * 512],
                                    lhsT=w_sb[nm][:, kt, ct * 128 : (ct + 1) * 128],
                                    rhs=h_sb[:, kt, ch * 512 : (ch + 1) * 512],
                                    start=(kt == 0),
                                    stop=(kt == CT - 1),
                                )
                        nc.vector.tensor_scalar(
                            out=dst[:, ct, :],
                            in0=ps,
                            scalar1=bias_sb[nm][:, ct : ct + 1],
                            scalar2=None,
                            op0=OP.add,
                        )
                for nt in range(NT):
                    ps = ps2.tile([128, N], F32, tag="ps2", name=f"ps_v{nt}")
                    for kt in range(CT):
                        nc.tensor.matmul(
                            out=ps[:, 0:512],
                            lhsT=h_sb[:, kt, nt * 128 : (nt + 1) * 128],
                            rhs=w_sb["v"][:, kt, :],
                            start=(kt == 0),
                            stop=(kt == CT - 1),
                        )
                    psv = ps[:, 0:512].rearrange("p (h d) -> p h d", d=D)
                    if apply_vb:
                        nc.vector.tensor_add(
                            out=vt_sb[:, nt, :, 0:D],
                            in0=psv,
                            in1=vb_bc.rearrange("p (h d) -> p h d", d=D),
                        )
                    else:
                        nc.vector.tensor_copy(out=vt_sb[:, nt, :, 0:D], in_=psv)

            def emit_C(b):
                """attention"""
                q_sb, k_sb, vt_sb = st[b]["q"], st[b]["k"], st[b]["vt"]
                att_sb = work.tile(
                    [128, CT, N], BF16, tag="att", bufs=1, name=f"att{b}"
                )
                st[b]["att"] = att_sb
                if dump and b == 0:
                    nc.gpsimd.dma_start(
                        out=dbg_ext.ap()[1][:, 0:4096],
                        in_=q_sb.rearrange("p a n -> p (a n)"),
                    )
                    nc.gpsimd.dma_start(
                        out=dbg_ext.ap()[2][:, 0:4096],
                        in_=k_sb.rearrange("p a n -> p (a n)"),
                    )
                    nc.gpsimd.dma_start(
                        out=dbg_ext.ap()[4][:, 0 : NT * NH * VG],
                        in_=vt_sb.rearrange("p a h g -> p (a h g)"),
                    )
                # compute-engine APs may only start at partition 0/32/64/96,
                # so Z rows land as columns of a single-partition tile and a
                # DMA respreads them across partitions for the reciprocal.
                zflat = small.tile([1, NH * N], BF16, tag="zflat", bufs=1)
                zrows = small.tile([NH, N], F32, tag="zrows", bufs=1)
                for hp in range(CT):
                    e_tiles = []
                    for mt in range(NT):
                        psS = ps1.tile([128, 2 * N], F32, tag="ps1", name=f"psS{hp}_{mt}")
                        e_t = epool.tile([128, 2, N], BF16, tag="e", name=f"e{hp}_{mt}")
                        for hi, p0 in ((0, 0), (1, 64)):
                            for ch in range(NCH):
                                nc.tensor.matmul(
                                    out=psS[
                                        :, hi * N + ch * 512 : hi * N + (ch + 1) * 512
                                    ],
                                    lhsT=k_sb[
                                        p0 : p0 + D, hp, mt * 128 : (mt + 1) * 128
                                    ],
                                    rhs=q_sb[p0 : p0 + D, hp, ch * 512 : (ch + 1) * 512],
                                    start=True,
                                    stop=True,
                                    tile_position=(p0, 0),
                                )
                        nc.scalar.activation(
                            out=e_t.rearrange("p a n -> p (a n)"),
                            in_=psS,
                            func=AF.Exp,
                            scale=0.125,
                        )
                        e_tiles.append(e_t)
                    for hi, p0 in ((0, 0), (1, 64)):
                        h_ = 2 * hp + hi
                        pso = ps2.tile([128, N], F32, tag="ps2", name=f"psO{hp}_{hi}")
                        for ch in range(NCH):
                            for mt in range(NT):
                                nc.tensor.matmul(
                                    out=pso[0 : D + 1, ch * 512 : (ch + 1) * 512],
                                    lhsT=vt_sb[:, mt, h_, 0 : D + 1],
                                    rhs=e_tiles[mt][:, hi, ch * 512 : (ch + 1) * 512],
                                    start=(mt == 0),
                                    stop=(mt == NT - 1),
                                )
                        nc.vector.tensor_copy(
                            out=att_sb[p0 : p0 + D, hp, :], in_=pso[0:D, :]
                        )
                        nc.vector.tensor_copy(
                            out=zflat[0:1, h_ * N : (h_ + 1) * N],
                            in_=pso[D : D + 1, :],
                        )
                nc.gpsimd.dma_start(out=zrows, in_=zflat)
                rzt = small.tile([NH, N], F32, tag="rzt", bufs=1)
                nc.vector.reciprocal_approx_fast(out=rzt, in_=zrows)
                nc.gpsimd.dma_start(out=zdram.ap()[b], in_=rzt)
                if dump and b == 0:
                    nc.sync.dma_start(out=dbg_ext.ap()[5][0:8, 0:N], in_=zrows)
                    nc.sync.dma_start(out=dbg_ext.ap()[7][0:8, 0:N], in_=rzt)
                    nc.gpsimd.dma_start(
                        out=dbg_ext.ap()[8][:, 0:4096],
                        in_=att_sb.rearrange("p a n -> p (a n)"),
                    )
                for hp in range(CT):
                    rzb = small.tile([128, N], BF16, tag="rzb")
                    for hi, p0 in ((0, 0), (1, 64)):
                        nc.sync.dma_start(
                            out=rzb[p0 : p0 + D, :],
                            in_=_bcast_rows(zdram.ap()[b][2 * hp + hi], D),
                        )
                    nc.vector.tensor_mul(
                        out=att_sb[:, hp, :], in0=att_sb[:, hp, :], in1=rzb
                    )
                if dump and b == 0:
                    nc.gpsimd.dma_start(
                        out=dbg_ext.ap()[3][:, 0:4096],
                        in_=att_sb.rearrange("p a n -> p (a n)"),
                    )

            def emit_D(b):
                """proj + residual + store"""
                x_sb, att_sb = st[b]["x"], st[b]["att"]
                ov = out_ext.ap()[b].rearrange("(ct p) n -> p ct n", p=128)
                for ct in range(CT):
                    ps = ps2.tile([128, N], F32, tag="ps2", name=f"ps_p{ct}")
                    for ch in range(NCH):
                        for kt in range(CT):
                            nc.tensor.matmul(
                                out=ps[:, ch * 512 : (ch + 1) * 512],
                                lhsT=w_sb["p"][:, kt, ct * 128 : (ct + 1) * 128],
                                rhs=att_sb[:, kt, ch * 512 : (ch + 1) * 512],
                                start=(kt == 0),
                                stop=(kt == CT - 1),
                            )
                    nc.vector.scalar_tensor_tensor(
                        out=x_sb[:, ct, :],
                        in0=ps,
                        scalar=bias_sb["p"][:, ct : ct + 1],
                        in1=x_sb[:, ct, :],
                        op0=OP.add,
                        op1=OP.add,
                    )
                    nc.sync.dma_start(out=ov[:, ct, :], in_=x_sb[:, ct, :])

            emit_A(0)
            emit_B(0)
            emit_A(1)
            emit_C(0)
            emit_B(1)
            emit_D(0)
            emit_C(1)
            emit_D(1)

    nc.compile()
    return nc


def kernel(x, norm_scale, norm_bias, q_w, q_b, k_w, k_b, v_w, v_b, proj_w, proj_b,
           _dump=False):
    x = np.ascontiguousarray(np.asarray(x, dtype=np.float32))
    b, c, hh, ww = x.shape
    assert (b, c, hh * ww) == (16, C, N)
    xr = x.reshape(b, c, hh * ww)

    wts = {
        "qwT": np.ascontiguousarray(np.asarray(q_w, np.float32).T),
        "kwT": np.ascontiguousarray(np.asarray(k_w, np.float32).T),
        "vwT": np.ascontiguousarray(np.asarray(v_w, np.float32).T),
        "pwT": np.ascontiguousarray(np.asarray(proj_w, np.float32).T),
        "qb": np.ascontiguousarray(np.asarray(q_b, np.float32)),
        "kb": np.ascontiguousarray(np.asarray(k_b, np.float32)),
        "vb": np.ascontiguousarray(np.asarray(v_b, np.float32)),
        "pb": np.ascontiguousarray(np.asarray(proj_b, np.float32)),
        "nsc": np.ascontiguousarray(np.asarray(norm_scale, np.float32)),
        "nbi": np.ascontiguousarray(np.asarray(norm_bias, np.float32)),
    }
    apply_vb = bool(np.any(wts["vb"]))

    nc = build_nc(apply_vb, dump=_dump)
    in_maps = []
    for i in range(N_CORES):
        m = dict(wts)
        m["x"] = np.ascontiguousarray(xr[i * B_PER_CORE : (i + 1) * B_PER_CORE])
        in_maps.append(m)

    res = run_bass_kernel_spmd(nc, in_maps, core_ids=list(range(N_CORES)))
    kernel.last_result = res
    out = np.concatenate([res.results[i]["out"] for i in range(N_CORES)], axis=0)
    return out.reshape(b, c, hh, ww).astype(np.float32)
